# revision 5
# baseline (speedup 1.0000x reference)
"""Trainium2 Bass kernel for the 2-layer GRU discriminator
(B=1024, T=63, F=257, H=512  ->  out [1024, 1]).

Strategy (pure data parallelism over batch, 8 cores x 128 batch each):
  - State kept as h [b=128 partitions, H free] bf16; per-step PE transposes
    produce hT (cast fp8) used as the stationary operand so gate matmuls run
    with SBUF-resident weights as the moving operand at N=512, fp8 DoubleRow
    (K=256 per matmul).
  - L0 x-contribution contracts only f=0..255 as one DR pair (3 matmuls);
    feature 256 + biases enter via K<=2 rank-1 matmuls packed 3-4x into the
    PE array with tile_position row groups (~1 matmul's cost for all).
    All five full-width bias matmuls of the old design are gone.
  - PSUM map (exactly 8 banks, no same-step pool rotation stalls):
    rz0[2] rz1[2] hn0[1] hn1[1] xn1[1] ptr[1]. L0's i_n is evacuated from
    hn0 to SBUF by an early ACT copy; the hn0 bank is then reused for the
    hidden-n accumulation (second group), making t4 an all-bf16 DVE op.
  - Gate algebra: h' = u - (z-1)*n with u = z*h_prev on the (idle) GpSimd
    engine and (z-1)*n as one scalar_tensor_tensor -- no 1-z sigmoid.
  - Head: out[b] = sum_t v_t*(c_t . dnn_w) via one fused DVE
    tensor_tensor_reduce per step into s_all[:, t]; final weighted reduce
    by v at the end. No PE involvement, no PSUM bank.
"""
import numpy as np
import ml_dtypes
from contextlib import ExitStack

import concourse.bass as bass
import concourse.tile as tile
from concourse import bacc, mybir
from concourse.bass_utils import run_bass_kernel_spmd

AF = mybir.ActivationFunctionType
OP = mybir.AluOpType
PM = mybir.MatmulPerfMode
AX = mybir.AxisListType
F32 = mybir.dt.float32
BF16 = mybir.dt.bfloat16
FP8 = mybir.dt.float8e4
NPBF = ml_dtypes.bfloat16
NPF8 = ml_dtypes.float8_e4m3

B, T, F, H = 1024, 63, 257, 512
NCORES = 8
BC = B // NCORES          # 128 batch per core
G3 = 3 * H                # 1536
NKQ = 2                   # hidden chunk-pairs (DoubleRow, K=256 each)
WSCALE = 16.0             # fp8 weight pre-scale (descaled in sigmoid/tanh)


def _dr(ap):
    """[128, 2*X] slice -> [128, 2, X] chunk-pair AP for DoubleRow."""
    return ap.rearrange("p (i b) -> p i b", i=2)


def _build_module():
    nc = bacc.Bacc("TRN2", target_bir_lowering=False, debug=False)

    xT_d = nc.dram_tensor("xT", [T, 128, 2 * BC], FP8, kind="ExternalInput").ap()
    xr_d = nc.dram_tensor("xr", [128, T * 128], FP8, kind="ExternalInput").ap()
    wm_d = nc.dram_tensor("wm", [128, H], FP8, kind="ExternalInput").ap()
    ones_d = nc.dram_tensor("ones8", [128, 128], FP8, kind="ExternalInput").ap()
    wb_d = nc.dram_tensor("wb", [128, H], FP8, kind="ExternalInput").ap()
    wih0_d = nc.dram_tensor("wih0", [1, 128, 2 * G3], FP8, kind="ExternalInput").ap()
    whh0_d = nc.dram_tensor("whh0", [NKQ, 128, 2 * G3], FP8, kind="ExternalInput").ap()
    wih1_d = nc.dram_tensor("wih1", [NKQ, 128, 2 * G3], FP8, kind="ExternalInput").ap()
    whh1_d = nc.dram_tensor("whh1", [NKQ, 128, 2 * G3], FP8, kind="ExternalInput").ap()
    iden_d = nc.dram_tensor("iden", [128, 128], BF16, kind="ExternalInput").ap()
    dnnb_d = nc.dram_tensor("dnnb", [128, H], BF16, kind="ExternalInput").ap()
    vrow_d = nc.dram_tensor("vrow", [128, 64], F32, kind="ExternalInput").ap()
    out_d = nc.dram_tensor("out", [128, 1], F32, kind="ExternalOutput").ap()

    with tile.TileContext(nc) as tc, ExitStack() as ctx:
        wp = ctx.enter_context(tc.tile_pool(name="wp", bufs=1, space="SBUF"))
        xp = ctx.enter_context(tc.tile_pool(name="xp", bufs=4, space="SBUF"))
        sp = ctx.enter_context(tc.tile_pool(name="sp", bufs=2, space="SBUF"))
        # PSUM pools: one tag per bank role, bufs=1 (8 banks total)
        prz0 = ctx.enter_context(tc.tile_pool(name="prz0", bufs=1, space="PSUM"))
        prz1 = ctx.enter_context(tc.tile_pool(name="prz1", bufs=1, space="PSUM"))
        phn0p = ctx.enter_context(tc.tile_pool(name="phn0p", bufs=1, space="PSUM"))
        phn1p = ctx.enter_context(tc.tile_pool(name="phn1p", bufs=1, space="PSUM"))
        pxn1p = ctx.enter_context(tc.tile_pool(name="pxn1p", bufs=1, space="PSUM"))
        ptrp = ctx.enter_context(tc.tile_pool(name="ptrp", bufs=1, space="PSUM"))

        # --- resident weights ---
        wih0 = wp.tile_from(wih0_d[0], name="wih0")
        whh0 = [wp.tile_from(whh0_d[q], name=f"whh0_{q}") for q in range(NKQ)]
        wih1 = [wp.tile_from(wih1_d[q], name=f"wih1_{q}") for q in range(NKQ)]
        whh1 = [wp.tile_from(whh1_d[q], name=f"whh1_{q}") for q in range(NKQ)]
        xr = wp.tile_from(xr_d, name="xr")
        wm = wp.tile_from(wm_d, name="wm")
        ones8 = wp.tile_from(ones_d, name="ones8")
        wb = wp.tile_from(wb_d, name="wb")
        iden = wp.tile_from(iden_d, name="iden")
        dnnb = wp.tile_from(dnnb_d, name="dnnb")
        vrow = wp.tile_from(vrow_d, name="vrow")
        s_all = wp.tile([128, 64], F32, name="s_all")
        nc.gpsimd.memset(s_all[:, T:64], 0.0)

        def wslice(wtile, g0, g1):
            """[128, 2*G3] pair tile -> [128, 2, g1-g0] moving operand."""
            return wtile.rearrange("p (i g) -> p i g", i=2)[:, :, g0:g1]

        # static PSUM transpose staging: aT in [:, :512], cT in [:, 512:]
        ptr = ptrp.tile([128, 1024], BF16, name="ptr", tag="ptr")

        aT = None
        cT = None
        a_prev = None
        c_prev = None

        def gru_gates(przt, phnt, pxn_ap, pxn_fast, prev, nm):
            """PSUM preacts (x WSCALE) -> new state [BC, H] bf16 in SBUF.
            przt: [128,1024] r|z merged tile; phnt: hidden-n PSUM;
            pxn_ap: input-n contribution (SBUF bf16 if pxn_fast else PSUM)."""
            r = sp.tile([BC, H], BF16, name=f"r_{nm}", tag=f"r_{nm}")
            z = sp.tile([BC, H], BF16, name=f"z_{nm}", tag=f"z_{nm}")
            nn = sp.tile([BC, H], BF16, name=f"n_{nm}", tag=f"n_{nm}")
            t3 = sp.tile([BC, H], BF16, name=f"t3_{nm}", tag=f"t3_{nm}")
            t4 = sp.tile([BC, H], BF16, name=f"t4_{nm}", tag=f"t4_{nm}")
            t6 = sp.tile([BC, H], BF16, name=f"t6_{nm}", tag=f"t6_{nm}")
            hnew = sp.tile([BC, H], BF16, name=f"h_{nm}", tag=f"h_{nm}")
            nc.scalar.activation(out=r, in_=przt[:, 0:H], func=AF.Sigmoid,
                                 scale=1.0 / WSCALE)
            nc.scalar.activation(out=z, in_=przt[:, H:2 * H], func=AF.Sigmoid,
                                 scale=1.0 / WSCALE)
            nc.vector.tensor_tensor(out=t3, in0=r, in1=phnt, op=OP.mult)
            nc.vector.tensor_tensor(out=t4, in0=t3, in1=pxn_ap, op=OP.add)
            nc.scalar.activation(out=nn, in_=t4, func=AF.Tanh, scale=1.0 / WSCALE)
            # (z-1)*n in one fused op; h' = u - (z-1)*n with u = z*prev
            nc.vector.scalar_tensor_tensor(out=t6, in0=z, scalar=1.0, in1=nn,
                                           op0=OP.subtract, op1=OP.mult)
            if prev is None:
                nc.vector.tensor_scalar_mul(hnew, t6, -1.0)
            else:
                u = sp.tile([BC, H], BF16, name=f"u_{nm}", tag=f"u_{nm}")
                nc.gpsimd.tensor_tensor(out=u, in0=z, in1=prev, op=OP.mult)
                nc.vector.tensor_tensor(out=hnew, in0=u, in1=t6, op=OP.subtract)
            return hnew

        def transpose_state(h, half, nm):
            """[BC, H] SBUF bf16 -> [128, H] SBUF fp8 (transposed chunks)."""
            off = half * H
            for k in range(4):
                nc.tensor.transpose(
                    out=ptr[:, off + k * 128:off + (k + 1) * 128],
                    in_=h[:, k * 128:(k + 1) * 128],
                    identity=iden,
                )
            hT = sp.tile([128, H], FP8, name=f"hT_{nm}", tag=f"hT_{nm}")
            nc.scalar.activation(out=hT[:, 0:256], in_=ptr[:, off:off + 256],
                                 func=AF.Copy)
            nc.vector.tensor_copy(out=hT[:, 256:512], in_=ptr[:, off + 256:off + H])
            return hT

        for t in range(T):
            # ---- stream x_t (pre-transposed f0..255 pair, fp8 on host) ----
            xt = xp.tile([128, 2 * BC], FP8, name="xt", tag="xt")
            nc.sync.dma_start(out=xt, in_=xT_d[t])
            xq = _dr(xt)
            xrs = lambda p: xr[p:p + 2, t * 128:(t + 1) * 128]

            # ---- PSUM tiles for this step ----
            rz0 = prz0.tile([BC, 2 * H], F32, name="rz0", tag="rz0")
            hn0 = phn0p.tile([BC, H], F32, name="hn0", tag="hn0")
            rz1 = prz1.tile([BC, 2 * H], F32, name="rz1", tag="rz1")
            hn1 = phn1p.tile([BC, H], F32, name="hn1", tag="hn1")
            xn1 = pxn1p.tile([BC, H], F32, name="xn1", tag="xn1")

            # ---- L1 bias group (4 packed rank-1 matmuls, step start) ----
            nc.tensor.matmul(rz1[:, 0:H], ones8[0:1, 0:128], wb[0:1, :],
                             start=True, stop=False, tile_position=(0, 0))
            nc.tensor.matmul(rz1[:, H:2 * H], ones8[32:33, 0:128], wb[32:33, :],
                             start=True, stop=False, tile_position=(32, 0))
            nc.tensor.matmul(hn1, ones8[64:65, 0:128], wb[64:65, :],
                             start=True, stop=(t == 0), tile_position=(64, 0))
            nc.tensor.matmul(xn1, ones8[96:97, 0:128], wb[96:97, :],
                             start=True, stop=False, tile_position=(96, 0))

            # ---- L1 hidden matmuls (cT from step t-1, ready at step start) ----
            if t > 0:
                for q in range(NKQ):
                    cq = _dr(cT[:, 2 * q * 128:(2 * q + 2) * 128])
                    nc.tensor.matmul(rz1[:, 0:H], cq, wslice(whh1[q], 0, H),
                                     start=False, stop=False,
                                     perf_mode=PM.DoubleRow)
                    nc.tensor.matmul(rz1[:, H:2 * H], cq, wslice(whh1[q], H, 2 * H),
                                     start=False, stop=False,
                                     perf_mode=PM.DoubleRow)
                    nc.tensor.matmul(hn1, cq, wslice(whh1[q], 2 * H, G3),
                                     start=False, stop=(q == NKQ - 1),
                                     perf_mode=PM.DoubleRow)

            # ---- L0 rank-1 group (x256 + biases; 3 packed matmuls) ----
            nc.tensor.matmul(rz0[:, 0:H], xrs(0), wm[0:2, :],
                             start=True, stop=False, tile_position=(0, 0))
            nc.tensor.matmul(rz0[:, H:2 * H], xrs(32), wm[32:34, :],
                             start=True, stop=False, tile_position=(32, 0))
            nc.tensor.matmul(hn0, xrs(64), wm[64:66, :],
                             start=True, stop=False, tile_position=(64, 0))

            # ---- L0 x matmuls (one DR pair, f0..255) ----
            nc.tensor.matmul(rz0[:, 0:H], xq, wslice(wih0, 0, H),
                             start=False, stop=(t == 0), perf_mode=PM.DoubleRow)
            nc.tensor.matmul(rz0[:, H:2 * H], xq, wslice(wih0, H, 2 * H),
                             start=False, stop=(t == 0), perf_mode=PM.DoubleRow)
            nc.tensor.matmul(hn0, xq, wslice(wih0, 2 * H, G3),
                             start=False, stop=True, perf_mode=PM.DoubleRow)

            # evacuate i_n to SBUF (frees hn0 bank for the hidden-n group)
            in0 = sp.tile([BC, H], BF16, name="in0", tag="in0")
            nc.scalar.activation(out=in0, in_=hn0, func=AF.Copy)

            # hidden-n second group in the same bank: b_hh0n + h.W_hhn
            nc.tensor.matmul(hn0, ones8[96:97, 0:128], wm[96:97, :],
                             start=True, stop=(t == 0), tile_position=(96, 0))
            if t > 0:
                for q in range(NKQ):
                    aq = _dr(aT[:, 2 * q * 128:(2 * q + 2) * 128])
                    nc.tensor.matmul(rz0[:, 0:H], aq, wslice(whh0[q], 0, H),
                                     start=False, stop=(q == NKQ - 1),
                                     perf_mode=PM.DoubleRow)
                    nc.tensor.matmul(rz0[:, H:2 * H], aq, wslice(whh0[q], H, 2 * H),
                                     start=False, stop=(q == NKQ - 1),
                                     perf_mode=PM.DoubleRow)
                    nc.tensor.matmul(hn0, aq, wslice(whh0[q], 2 * H, G3),
                                     start=False, stop=(q == NKQ - 1),
                                     perf_mode=PM.DoubleRow)

            a_new = gru_gates(rz0, hn0, in0, True, a_prev, "a")
            aT = transpose_state(a_new, 0, "a")
            a_prev = a_new

            # ---- L1 input matmuls (aT of this step) ----
            for q in range(NKQ):
                aq = _dr(aT[:, 2 * q * 128:(2 * q + 2) * 128])
                nc.tensor.matmul(rz1[:, 0:H], aq, wslice(wih1[q], 0, H),
                                 start=False, stop=(q == NKQ - 1),
                                 perf_mode=PM.DoubleRow)
                nc.tensor.matmul(rz1[:, H:2 * H], aq, wslice(wih1[q], H, 2 * H),
                                 start=False, stop=(q == NKQ - 1),
                                 perf_mode=PM.DoubleRow)
                nc.tensor.matmul(xn1, aq, wslice(wih1[q], 2 * H, G3),
                                 start=False, stop=(q == NKQ - 1),
                                 perf_mode=PM.DoubleRow)

            c_new = gru_gates(rz1, hn1, xn1, False, c_prev, "c")
            cT = transpose_state(c_new, 1, "c")
            c_prev = c_new

            # ---- head contribution: s_all[:, t] = c_new . dnn_w ----
            hm = sp.tile([BC, H], BF16, name="hm", tag="hm")
            nc.vector.scalar_tensor_tensor(
                out=hm, in0=c_new, scalar=1.0, in1=dnnb,
                op0=OP.mult, op1=OP.mult, accum_out=s_all[:, t:t + 1])

        # ---- final head combine: out[b] = sum_t v_t * s_all[b, t] ----
        sw = sp.tile([128, 64], F32, name="sw", tag="sw")
        acc = sp.tile([128, 1], F32, name="acc", tag="acc")
        nc.vector.scalar_tensor_tensor(
            out=sw, in0=s_all, scalar=1.0, in1=vrow,
            op0=OP.mult, op1=OP.mult, accum_out=acc)
        nc.sync.dma_start(out=out_d, in_=acc)

    # legalize sem waits (>=2 waits per matmul is a codegen error) etc.
    nc.compile()
    return nc


def _pack_pairs(wt):
    """[K, G3] (contraction-major, K multiple of 256) -> [K//256, 128, 2*G3]:
    out[q][p, i*G3+g] = wt[(2q+i)*128 + p, g]"""
    nq = wt.shape[0] // 256
    return np.ascontiguousarray(
        wt.reshape(nq, 2, 128, -1).transpose(0, 2, 1, 3).reshape(nq, 128, -1))


def host_prep(inputs):
    f32 = np.float32
    x = np.asarray(inputs["x"], f32)
    w_ih0, w_hh0 = np.asarray(inputs["w_ih0"], f32), np.asarray(inputs["w_hh0"], f32)
    b_ih0, b_hh0 = np.asarray(inputs["b_ih0"], f32), np.asarray(inputs["b_hh0"], f32)
    w_ih1, w_hh1 = np.asarray(inputs["w_ih1"], f32), np.asarray(inputs["w_hh1"], f32)
    b_ih1, b_hh1 = np.asarray(inputs["b_ih1"], f32), np.asarray(inputs["b_hh1"], f32)
    dnn_w, dnn_b = np.asarray(inputs["dnn_w"], f32), np.asarray(inputs["dnn_b"], f32)
    w1, b1 = np.asarray(inputs["w1"], f32), np.asarray(inputs["b1"], f32)
    w2, b2 = np.asarray(inputs["w2"], f32), np.asarray(inputs["b2"], f32)
    w3, b3 = np.asarray(inputs["w3"], f32), np.asarray(inputs["b3"], f32)

    wih0 = _pack_pairs(w_ih0.T[:256] * WSCALE).astype(NPF8)
    whh0 = _pack_pairs(w_hh0.T * WSCALE).astype(NPF8)
    wih1 = _pack_pairs(w_ih1.T * WSCALE).astype(NPF8)
    whh1 = _pack_pairs(w_hh1.T * WSCALE).astype(NPF8)

    b0g = (b_ih0 + b_hh0) * WSCALE
    b1g = (b_ih1 + b_hh1) * WSCALE
    # rank-1 moving rows: x256 weight rows + L0 biases
    wm = np.zeros((128, H), f32)
    wm[0] = w_ih0[0:H, 256] * WSCALE
    wm[1] = b0g[0:H]
    wm[32] = w_ih0[H:2 * H, 256] * WSCALE
    wm[33] = b0g[H:2 * H]
    wm[64] = w_ih0[2 * H:G3, 256] * WSCALE
    wm[65] = b_ih0[2 * H:G3] * WSCALE
    wm[96] = b_hh0[2 * H:G3] * WSCALE
    wm = wm.astype(NPF8)
    # L1 bias moving rows
    wb = np.zeros((128, H), f32)
    wb[0] = b1g[0:H]
    wb[32] = b1g[H:2 * H]
    wb[64] = b_hh1[2 * H:G3] * WSCALE
    wb[96] = b_ih1[2 * H:G3] * WSCALE
    wb = wb.astype(NPF8)

    ones8 = np.zeros((128, 128), NPF8)
    for p in (0, 32, 64, 96):
        ones8[p] = 1.0

    v = (w3 @ w2 @ w1)[0]
    dnnb = np.broadcast_to(dnn_w[0], (128, H)).astype(NPBF)
    vrow = np.zeros((128, 64), f32)
    vrow[:, :T] = v[None, :]
    c_all = float(v.sum() * dnn_b[0] + (w3 @ w2 @ b1)[0] + (w3 @ b2)[0] + b3[0])

    shared = dict(
        wih0=wih0, whh0=whh0, wih1=wih1, whh1=whh1, wm=wm, wb=wb,
        ones8=ones8, iden=np.eye(128, dtype=NPBF), dnnb=dnnb,
        vrow=np.ascontiguousarray(vrow))

    percore = []
    for c in range(NCORES):
        xc = x[c * BC:(c + 1) * BC]                       # [BC, T, F]
        xmain = xc[:, :, :256]                            # [BC, T, 256]
        xT = xmain.reshape(BC, T, 2, 128).transpose(1, 3, 2, 0)
        xT = np.ascontiguousarray(xT.reshape(T, 128, 2 * BC)).astype(NPF8)
        xr = np.zeros((128, T * 128), f32)
        x256 = xc[:, :, 256]                              # [BC, T]
        for p in (0, 32, 64):
            xr[p] = x256.T.reshape(-1)                    # x256[b,t] at t*128+b
            xr[p + 1] = 1.0
        xr[96] = 1.0
        percore.append(dict(xT=xT, xr=xr.astype(NPF8)))
    return shared, percore, c_all


_CACHED = {}


def _get_module():
    if "nc" not in _CACHED:
        _CACHED["nc"] = _build_module()
    return _CACHED["nc"]


def kernel(**inputs) -> np.ndarray:
    shared, percore, c_all = host_prep(inputs)
    nc = _get_module()
    in_maps = [{**shared, **percore[c]} for c in range(NCORES)]
    res = run_bass_kernel_spmd(nc, in_maps, core_ids=list(range(NCORES)))
    outs = [res.results[c]["out"].reshape(BC) for c in range(NCORES)]
    out = np.concatenate(outs).astype(np.float32) + np.float32(c_all)
    return out.reshape(B, 1)
